# revision 5
# baseline (speedup 1.0000x reference)
"""CBOW negative-sampling loss on 8 Trainium2 NeuronCores.

Strategy (from sharding hint): replicate both embedding tables, data-parallel
over the batch dim. Each core handles 2048 of the 16384 batch rows.

Per-core kernel layout:
  - batch row b -> chunk c = b // 128, partition p = b % 128.
  - 16 chunks, processed in 4 groups of 4 chunks; each group does ONE
    indirect-DMA gather from u_emb (128 part x 32 idx x 128 f32) and ONE from
    w_emb (128 part x 24 idx x 128 f32), amortizing SWDGE fixed overhead.
  - per chunk: h = sum of 8 context embeddings (DVE reduce over strided AP),
    scores = per-row dot(h, w_gathered) for [pos, 5 negs] via broadcast-mult +
    reduce; pos score negated in the reduce.
  - Sigmoid then Ln (with accum_out) over all 96 scores/partition gives the
    per-partition sum of log_sigmoid terms [128, 1]; host sums 8x128 partials
    and negates.

loss = -[ sum_b ln(sigmoid(score_b)) + sum_{b,k} ln(sigmoid(-neg_score_bk)) ]
"""

import sys

import numpy as np

sys.path.insert(0, "/opt/trn_rl_repo")

from concourse import bacc, bass, mybir, tile  # noqa: E402
from concourse.bass_utils import run_bass_kernel_spmd  # noqa: E402

V, D = 100000, 128
B, C, K = 16384, 8, 5
N_CORES = 8
P = 128
B_LOC = B // N_CORES            # 2048 batch rows per core
N_CHUNK = B_LOC // P            # 16 chunks of 128 rows
GROUP = 4                       # chunks per indirect-DMA gather
N_GROUP = N_CHUNK // GROUP      # 4
UW = GROUP * C                  # 32 u-indices per partition per group
J = 1 + K                       # 6 w-rows per batch row (pos + negs)
WW = GROUP * J                  # 24 w-indices per partition per group
NS = N_CHUNK * J                # 96 score columns per partition

_NC_CACHE = {}


def _build_bass():
    nc = bacc.Bacc("TRN2", target_bir_lowering=False, debug=False)

    u_emb = nc.dram_tensor("u_emb", [V, D], mybir.dt.float32, kind="ExternalInput")
    w_emb = nc.dram_tensor("w_emb", [V, D], mybir.dt.float32, kind="ExternalInput")
    uidx = nc.dram_tensor("uidx", [N_GROUP, P, UW], mybir.dt.int32, kind="ExternalInput")
    widx = nc.dram_tensor("widx", [N_GROUP, P, WW], mybir.dt.int32, kind="ExternalInput")
    loss = nc.dram_tensor("loss_part", [P, 1], mybir.dt.float32, kind="ExternalOutput")

    fp32 = mybir.dt.float32
    X = mybir.AxisListType.X
    ADD = mybir.AluOpType.add

    with tile.TileContext(nc) as tc:
        with (
            tc.tile_pool(name="idx", bufs=2) as idx_pool,
            tc.tile_pool(name="ug", bufs=2) as ug_pool,
            tc.tile_pool(name="wg", bufs=2) as wg_pool,
            tc.tile_pool(name="h", bufs=2) as h_pool,
            tc.tile_pool(name="m", bufs=2) as m_pool,
            tc.tile_pool(name="fin", bufs=1) as fin_pool,
        ):
            scores = fin_pool.tile([P, NS], fp32, tag="scores")
            for g in range(N_GROUP):
                uix = idx_pool.tile([P, UW], mybir.dt.int32, tag="uix")
                wix = idx_pool.tile([P, WW], mybir.dt.int32, tag="wix")
                nc.sync.dma_start(out=uix[:], in_=uidx[g])
                nc.sync.dma_start(out=wix[:], in_=widx[g])

                ug = ug_pool.tile([P, UW * D], fp32, tag="ug")
                wg = wg_pool.tile([P, WW * D], fp32, tag="wg")
                nc.gpsimd.indirect_dma_start(
                    out=ug[:],
                    out_offset=None,
                    in_=u_emb[:],
                    in_offset=bass.IndirectOffsetOnAxis(ap=uix[:], axis=0),
                )
                nc.gpsimd.indirect_dma_start(
                    out=wg[:],
                    out_offset=None,
                    in_=w_emb[:],
                    in_offset=bass.IndirectOffsetOnAxis(ap=wix[:], axis=0),
                )

                for c in range(GROUP):
                    chunk = g * GROUP + c
                    # h = sum over the 8 context embeddings (reduce innermost
                    # after an AP permute that makes k the innermost axis)
                    h = h_pool.tile([P, D], fp32, tag="h")
                    u_view = ug[:, c * C * D : (c + 1) * C * D].rearrange(
                        "p (k d) -> p d k", k=C
                    )
                    nc.vector.tensor_reduce(out=h[:], in_=u_view, axis=X, op=ADD)

                    # m[p, j, d] = w_gathered[p, j, d] * h[p, d]
                    m = m_pool.tile([P, J * D], fp32, tag="m")
                    w_view = wg[:, c * J * D : (c + 1) * J * D].rearrange(
                        "p (j d) -> p j d", j=J
                    )
                    m_view = m[:].rearrange("p (j d) -> p j d", j=J)
                    nc.vector.tensor_mul(
                        out=m_view,
                        in0=w_view,
                        in1=h[:, None, :].broadcast_to([P, J, D]),
                    )

                    # scores[:, 6*chunk]     = +dot(h, w_pos)
                    # scores[:, 6*chunk+1:6] = -dot(h, w_negk)
                    # so each entry x contributes log_sigmoid(x) to the
                    # (negated) loss; host computes loss = -sum(ln(sigmoid(x)))
                    s0 = J * chunk
                    nc.vector.tensor_reduce(
                        out=scores[:, s0 : s0 + 1],
                        in_=m_view[:, 0:1, :],
                        axis=X,
                        op=ADD,
                    )
                    nc.vector.tensor_reduce(
                        out=scores[:, s0 + 1 : s0 + J],
                        in_=m_view[:, 1:J, :],
                        axis=X,
                        op=ADD,
                        negate=True,
                    )

            sg = fin_pool.tile([P, NS], fp32, tag="sg")
            sp = fin_pool.tile([P, NS], fp32, tag="sp")
            lp = fin_pool.tile([P, 1], fp32, tag="lp")
            nc.scalar.activation(
                out=sg[:],
                in_=scores[:],
                func=mybir.ActivationFunctionType.Sigmoid,
            )
            nc.scalar.activation(
                out=sp[:],
                in_=sg[:],
                func=mybir.ActivationFunctionType.Ln,
                accum_out=lp[:],
            )
            nc.sync.dma_start(out=loss[:], in_=lp[:])

    nc.compile()
    return nc


def _get_nc():
    if "nc" not in _NC_CACHE:
        _NC_CACHE["nc"] = _build_bass()
    return _NC_CACHE["nc"]


def _make_in_maps(pos_u, pos_w, neg_w, u_emb, w_emb):
    pos_u = np.ascontiguousarray(np.asarray(pos_u).astype(np.int32))
    pos_w = np.ascontiguousarray(np.asarray(pos_w).astype(np.int32))
    neg_w = np.ascontiguousarray(np.asarray(neg_w).astype(np.int32))
    u_emb = np.ascontiguousarray(np.asarray(u_emb, dtype=np.float32))
    w_emb = np.ascontiguousarray(np.asarray(w_emb, dtype=np.float32))

    in_maps = []
    for i in range(N_CORES):
        sl = slice(i * B_LOC, (i + 1) * B_LOC)
        pu = pos_u[sl]                                        # [2048, 8]
        wi = np.concatenate([pos_w[sl, None], neg_w[sl]], 1)  # [2048, 6]
        # batch row b -> (group g, sub-chunk c, partition p): b = (4g+c)*128+p
        uidx = (
            pu.reshape(N_GROUP, GROUP, P, C).transpose(0, 2, 1, 3).reshape(N_GROUP, P, UW)
        )
        widx = (
            wi.reshape(N_GROUP, GROUP, P, J).transpose(0, 2, 1, 3).reshape(N_GROUP, P, WW)
        )
        in_maps.append(
            {
                "u_emb": u_emb,
                "w_emb": w_emb,
                "uidx": np.ascontiguousarray(uidx),
                "widx": np.ascontiguousarray(widx),
            }
        )
    return in_maps


def _install_axon_profile_shim():
    """Provide antenv.axon_hooks (missing in this image) so trace=True can
    capture NTFF profiles via the axon PJRT .so, and keep trace artifacts
    local instead of uploading to a bucket."""
    import contextlib
    import ctypes
    import types

    import concourse.bass_utils as bu

    bu.upload_artifacts = lambda tmpdir: tmpdir

    try:
        from antenv.axon_hooks import get_axon_ntff_profile_hook  # noqa: F401

        return
    except ImportError:
        pass

    mod = types.ModuleType("antenv.axon_hooks")
    holder = {}
    mod.set_axon_ntff_profile_hook = lambda h: holder.__setitem__("h", h)
    mod.get_axon_ntff_profile_hook = lambda: holder.get("h")
    sys.modules["antenv.axon_hooks"] = mod
    import antenv

    antenv.axon_hooks = mod

    so_path = "/opt/axon/libaxon_pjrt.so"
    lib = ctypes.CDLL(so_path)
    if not hasattr(lib, "axon_start_nrt_profile"):
        return
    lib.axon_start_nrt_profile.argtypes = [
        ctypes.POINTER(ctypes.c_int64),
        ctypes.c_size_t,
    ]
    lib.axon_start_nrt_profile.restype = ctypes.c_int64
    lib.axon_stop_nrt_profile.argtypes = [ctypes.c_char_p]
    lib.axon_stop_nrt_profile.restype = ctypes.c_int64

    @contextlib.contextmanager
    def _hook(output_dir, device_ids):
        import jax

        jax.devices()
        if device_ids:
            ids = (ctypes.c_int64 * len(device_ids))(*device_ids)
            rc = lib.axon_start_nrt_profile(ids, len(device_ids))
        else:
            rc = lib.axon_start_nrt_profile(None, 0)
        if rc != 0:
            raise RuntimeError(f"axon_start_nrt_profile rc={rc}")
        try:
            yield
        finally:
            n = lib.axon_stop_nrt_profile(str(output_dir).encode())
            print(f"profile: {n} file(s) written to {output_dir}")

    mod.set_axon_ntff_profile_hook(_hook)


def _run(in_maps, trace=False):
    if trace:
        _install_axon_profile_shim()
    nc = _get_nc()
    return run_bass_kernel_spmd(nc, in_maps, list(range(N_CORES)), trace=trace)


def kernel(pos_u, pos_w, neg_w, u_emb, w_emb):
    in_maps = _make_in_maps(pos_u, pos_w, neg_w, u_emb, w_emb)
    bkr = _run(in_maps, trace=False)
    total = 0.0
    for r in bkr.results:
        total += float(r["loss_part"].astype(np.float64).sum())
    return np.float32(-total)


def kernel_traced(pos_u, pos_w, neg_w, u_emb, w_emb):
    """Like kernel() but returns (loss, BassKernelResults) with HW profile."""
    in_maps = _make_in_maps(pos_u, pos_w, neg_w, u_emb, w_emb)
    bkr = _run(in_maps, trace=True)
    total = 0.0
    for r in bkr.results:
        total += float(r["loss_part"].astype(np.float64).sum())
    return np.float32(-total), bkr
